# revision 3
# baseline (speedup 1.0000x reference)
"""BiLSTM (reference nn_CharBiGRU) Trainium2 Bass kernel, v3.

Strategy (8 cores = 2 directions x 4 batch-quarters of 16 rows; each
core runs its 16 rows as FOUR independent scans of 4 rows, interleaved):
  - All per-step engine costs are free-dim bound, not batch bound, so
    4-row scans cost the same per op as 16-row ones — but four
    independent recurrence chains pipeline across PE/ACT/DVE/GPSIMD,
    hiding each chain's serial latency and keeping the PE busy enough
    to stay HAM-warm (2.4 GHz).
  - Phase 1: Z = x @ Wi.T + b for ALL timesteps as big bf16 GEMMs,
    staged to DRAM scratch in bf16.
  - Phase 2 per scan-step: gates in PSUM as [100, 512] (partition strip
    32s:32s+4 = h-block s; free = gate-major [i|f|o|g] x 128). Z_t
    enters via a K=4 identity matmul; h @ Wh.T accumulates as 16 bf16
    matmuls, 4-way column-tiled (tile_position=(0,32s)).
  - One sigmoid covers all gates (g pre-scaled x2: tanh = 2*sig(2x)-1,
    fixed on gpsimd); DVE does the cell update; one PE transpose of
    h [100,128] -> hT4 [128,100] yields next-step stationaries.
  - Backward direction runs on cores 4-7 over host-rotated x (same
    involution as the reference); host un-rotates its output.
"""

import numpy as np
from ml_dtypes import bfloat16

B, T, D, H = 64, 512, 512, 512
G4 = 4 * H
NCORES = 8
BL = 16   # batch rows per core
NS = 4    # independent scans per core
SB = BL // NS  # batch rows per scan (4)
GPERM = [0, 1, 3, 2]  # device gate order i,f,o,g ; reference is i,f,g,o

_CACHE = {}


def build_kernel(T_steps=T):
    import concourse.bass as bass
    import concourse.bacc as bacc
    import concourse.mybir as mybir
    from concourse.tile import TileContext
    from concourse.masks import make_identity

    fp32 = mybir.dt.float32
    bf16 = mybir.dt.bfloat16
    AF = mybir.ActivationFunctionType
    ALU = mybir.AluOpType

    TB = T_steps * SB       # (t, b) rows per scan
    MT = TB // 128          # phase-1 m-tiles per scan
    NP = 32 * 3 + SB        # 100 used partitions

    nc = bacc.Bacc()
    xT = nc.declare_dram_parameter("xT", [NS, 4, 128, TB], bf16, isOutput=False)
    wit = nc.declare_dram_parameter("wit", [4, 128, G4], bf16, isOutput=False)
    wht = nc.declare_dram_parameter("wht", [4, 128, G4], bf16, isOutput=False)
    brow = nc.declare_dram_parameter("brow", [1, G4], bf16, isOutput=False)
    h0t = nc.declare_dram_parameter("h0t", [NS, 128, NP], bf16, isOutput=False)
    c0l = nc.declare_dram_parameter("c0l", [NS, NP, 128], fp32, isOutput=False)
    ys = nc.declare_dram_parameter("ys", [NS, T_steps, 4, SB, 128], fp32,
                                   isOutput=True)
    Z = nc.dram_tensor("zscratch", [NS, TB, G4], bf16, kind="Internal")

    with TileContext(nc) as tc:
        with (
            tc.tile_pool(name="const", bufs=1) as constp,
            tc.tile_pool(name="wpool", bufs=1) as wpool,
            tc.tile_pool(name="state", bufs=1) as statep,
        ):
            identT = constp.tile([NP, NP], fp32)
            make_identity(nc, identT[:, :])
            identI = constp.tile([SB, SB], bf16)
            make_identity(nc, identI[:, :])
            ones1 = constp.tile([1, 128], bf16)
            nc.gpsimd.memset(ones1[:, :], 1.0)
            browsb = constp.tile([1, G4], bf16)
            nc.sync.dma_start(out=browsb[:, :], in_=brow[:, :])

            whsb = [wpool.tile([128, G4], bf16, tag=f"wh{k}", name=f"wh{k}")
                    for k in range(4)]
            wisb = [wpool.tile([128, G4], bf16, tag=f"wi{k}", name=f"wi{k}")
                    for k in range(4)]
            for k in range(4):
                nc.sync.dma_start(out=whsb[k][:, :], in_=wht[k])
                nc.sync.dma_start(out=wisb[k][:, :], in_=wit[k])

            # Per-scan state: hT4 cols 32k:32k+SB = stationary for h-block
            # k; C rows 32s:32s+SB = c for h-block s (gap rows inert).
            hT4 = [statep.tile([128, NP], bf16, tag=f"hT4_{j}", name=f"hT4_{j}")
                   for j in range(NS)]
            C = [statep.tile([NP, 128], fp32, tag=f"C_{j}", name=f"C_{j}")
                 for j in range(NS)]
            for j in range(NS):
                nc.sync.dma_start(out=hT4[j][:, :], in_=h0t[j])
                nc.sync.dma_start(out=C[j][:, :], in_=c0l[j])

            # ---- Phase 1: Z = x @ Wi.T + b (all t, all scans) ----
            with (
                tc.tile_pool(name="xin", bufs=4) as xinp,
                tc.tile_pool(name="zps", bufs=2, space="PSUM") as zpsp,
                tc.tile_pool(name="zst", bufs=3) as zstp,
            ):
                for j in range(NS):
                    for m in range(MT):
                        xk = [xinp.tile([128, 128], bf16, tag=f"x{k}",
                                        name=f"x{k}") for k in range(4)]
                        for k in range(4):
                            nc.sync.dma_start(
                                out=xk[k][:, :],
                                in_=xT[j, k, :, 128 * m:128 * (m + 1)])
                        zp = zpsp.tile([128, G4], fp32, tag="zp")
                        for s in range(4):
                            sl = slice(512 * s, 512 * (s + 1))
                            nc.tensor.matmul(zp[:, sl], ones1[0:1, :],
                                             browsb[0:1, sl],
                                             start=True, stop=False)
                            for k in range(4):
                                nc.tensor.matmul(zp[:, sl], xk[k][:, :],
                                                 wisb[k][:, sl],
                                                 start=False, stop=(k == 3))
                        zs = zstp.tile([128, G4], bf16, tag="zs")
                        if m % 2 == 0:
                            nc.vector.tensor_copy(zs[:, :], zp[:, :])
                        else:
                            nc.scalar.copy(zs[:, :], zp[:, :])
                        nc.sync.dma_start(out=Z[j, 128 * m:128 * (m + 1), :],
                                          in_=zs[:, :])

            # ---- Phase 2: four interleaved recurrent scans ----
            with (
                tc.tile_pool(name="zin", bufs=3) as zinp,
                tc.tile_pool(name="gps", bufs=1, space="PSUM") as gpsp,
                tc.tile_pool(name="ptp", bufs=1, space="PSUM") as ptp,
                tc.tile_pool(name="work", bufs=2) as workp,
            ):
                for t in range(T_steps):
                    for j in range(NS):
                        zsb = zinp.tile([SB, G4], bf16, tag=f"zsb{j}")
                        nc.sync.dma_start(out=zsb[:, :],
                                          in_=Z[j, t * SB:(t + 1) * SB, :])

                        G = gpsp.tile([NP, 512], fp32, tag=f"G{j}")
                        for s in range(4):
                            nc.tensor.matmul(
                                G[32 * s:32 * s + SB, :], identI[:, :],
                                zsb[:, 512 * s:512 * (s + 1)],
                                start=True, stop=False,
                                tile_position=(0, 32 * s),
                                skip_group_check=True)
                        for k in range(4):
                            for s in range(4):
                                nc.tensor.matmul(
                                    G[32 * s:32 * s + SB, :],
                                    hT4[j][:, 32 * k:32 * k + SB],
                                    whsb[k][:, 512 * s:512 * (s + 1)],
                                    start=False, stop=(k == 3),
                                    tile_position=(0, 32 * s),
                                    skip_group_check=True)

                        A = workp.tile([NP, 512], fp32, tag=f"A{j}")
                        nc.scalar.activation(A[:, :], G[0:NP, :], AF.Sigmoid)
                        nc.gpsimd.tensor_scalar(A[:, 384:512], A[:, 384:512],
                                                2.0, -1.0, ALU.mult, ALU.add)
                        T2 = workp.tile([NP, 128], fp32, tag=f"T2{j}")
                        nc.vector.tensor_mul(T2[:, :], A[:, 128:256], C[j][:, :])
                        T1 = workp.tile([NP, 128], fp32, tag=f"T1{j}")
                        nc.vector.tensor_mul(T1[:, :], A[:, 0:128],
                                             A[:, 384:512])
                        nc.vector.tensor_add(C[j][:, :], T1[:, :], T2[:, :])
                        TC = workp.tile([NP, 128], fp32, tag=f"TC{j}")
                        nc.scalar.activation(TC[:, :], C[j][:, :], AF.Tanh)
                        hsb = workp.tile([NP, 128], fp32, tag=f"hsb{j}")
                        nc.gpsimd.tensor_mul(hsb[:, :], A[:, 256:384], TC[:, :])

                        for s in range(4):
                            nc.sync.dma_start(
                                out=ys[j, t, s],
                                in_=hsb[32 * s:32 * s + SB, :])

                        PT = ptp.tile([128, NP], fp32, tag=f"PT{j}")
                        nc.tensor.transpose(PT[:, :], hsb[0:NP, :],
                                            identT[:, :])
                        nc.vector.tensor_copy(hT4[j][:, :], PT[:, :])

    nc.finalize()
    return nc


def _prep_w(Wi, Wh, b):
    """Reference (4H,K) weights -> device [4,128,4H] bf16 transposed chunks
    with columns ordered (h-block s, gate i/f/o/g, jj) and g scaled x2."""
    def cols(W):
        W = np.asarray(W, np.float32).reshape(4, 4, 128, -1)  # [gref, s, jj, K]
        W = W[GPERM]                 # -> device gate order i,f,o,g
        W[3] *= 2.0                  # g pre-scale for tanh-via-sigmoid
        W = W.transpose(1, 0, 2, 3).reshape(G4, -1)  # [(s,gd,jj), K]
        Wt = np.ascontiguousarray(W.T)               # [K, 4H]
        return Wt.reshape(4, 128, G4).astype(bfloat16)

    bv = np.asarray(b, np.float32).reshape(4, 4, 128)[GPERM]
    bv[3] *= 2.0
    bv = bv.transpose(1, 0, 2).reshape(1, G4).astype(bfloat16)
    return cols(Wi), cols(Wh), bv


def _host_prep(inputs_emb, mask, h0, c0, Wi_f, Wh_f, b_f, Wi_b, Wh_b, b_b):
    x = np.asarray(inputs_emb, dtype=np.float32)
    mask = np.asarray(mask, dtype=np.float32)
    lengths = mask.astype(np.int32).sum(axis=1)
    t_idx = np.arange(T, dtype=np.int64)[None, :]
    P = (lengths[:, None].astype(np.int64) - 1 - t_idx) % T  # involution
    x_proc = np.take_along_axis(x, P[:, :, None], axis=1)

    wif, whf, bf_ = _prep_w(Wi_f, Wh_f, b_f)
    wib, whb, bb_ = _prep_w(Wi_b, Wh_b, b_b)
    h0 = np.asarray(h0, np.float32)
    c0 = np.asarray(c0, np.float32)
    NP = 32 * 3 + SB

    in_maps = []
    for cidx in range(NCORES):
        d = cidx // 4
        base = (cidx % 4) * BL
        xd = (x if d == 0 else x_proc)
        xTa = np.zeros((NS, 4, 128, T * SB), np.float32)
        h0a = np.zeros((NS, 128, NP), np.float32)
        c0a = np.zeros((NS, NP, 128), np.float32)
        for j in range(NS):
            sl = slice(base + SB * j, base + SB * (j + 1))
            # xT[j, k, :, t*SB + b] = xd[b, t, 128k:...]
            xTa[j] = xd[sl].transpose(2, 1, 0).reshape(4, 128, T * SB)
            for k in range(4):
                h0a[j, :, 32 * k:32 * k + SB] = h0[sl, 128 * k:128 * (k + 1)].T
                c0a[j, 32 * k:32 * k + SB, :] = c0[sl, 128 * k:128 * (k + 1)]
        in_maps.append({
            "xT": xTa.astype(bfloat16),
            "wit": wif if d == 0 else wib,
            "wht": whf if d == 0 else whb,
            "brow": bf_ if d == 0 else bb_,
            "h0t": h0a.astype(bfloat16),
            "c0l": c0a,
        })
    return in_maps, P


def _host_post(results, P):
    outs = []
    for r in results:
        y = r["ys"]  # [NS, T, 4, SB, 128]
        Tn = y.shape[1]
        # -> [NS*SB, T, H]
        outs.append(np.ascontiguousarray(
            y.transpose(0, 3, 1, 2, 4)).reshape(BL, Tn, H))
    ys_f = np.concatenate(outs[:4], 0)  # [B, T, H]
    ys_b = np.concatenate(outs[4:], 0)
    out_b = np.take_along_axis(ys_b, P[:, :, None], axis=1)
    return np.concatenate([ys_f, out_b], axis=-1).astype(np.float32)


def kernel(**inputs):
    from concourse.bass_utils import run_bass_kernel_spmd
    in_maps, P = _host_prep(**inputs)
    if "nc" not in _CACHE:
        _CACHE["nc"] = build_kernel()
    nc = _CACHE["nc"]
    res = run_bass_kernel_spmd(nc, in_maps, list(range(NCORES)))
    return _host_post(res.results, P)


# revision 8
# speedup vs baseline: 3.2453x; 3.2453x over previous
"""BiLSTM (reference nn_CharBiGRU) Trainium2 Bass kernel, v2.

Strategy (8 cores = 2 directions x 4 batch-quarters, 16 rows each):
  - Phase 1: input projection Z = x @ Wi.T + b for ALL timesteps as one
    big bf16 GEMM (128-row (t,b) tiles, K=512, N=2048), staged to DRAM
    scratch in bf16.
  - Phase 2: per-step recurrence. Gates live in PSUM as [112, 512]:
    partition strip 32s:32s+16 = h-block s (j in [128s,128s+128)), free
    = gate-major [i|f|o|g] x 128 within the block. Z_t enters PSUM via a
    K=16 identity matmul; h @ Wh.T accumulates as 16 bf16 matmuls
    4-way column-tiled (tile_position=(0,32s)) so the four strips
    stream concurrently on separate XBUSes.
  - Elementwise keeps everything at matching partition bases: one
    sigmoid over all four gates (g pre-scaled by 2 on host so
    tanh(x) = 2*sigmoid(2x)-1), gpsimd fixes g, DVE does the cell
    update, one PE transpose of h [112,128] -> hT4 [128,112] yields all
    four next-step stationaries as column slices.
  - Backward direction runs on cores 4-7 over host-rotated x (same
    involution trick as the reference); host un-rotates its output.
"""

import numpy as np
from ml_dtypes import bfloat16

B, T, D, H = 64, 512, 512, 512
G4 = 4 * H
NCORES = 8
BL = 16  # batch rows per core
GPERM = [3, 0, 1, 2]  # device gate order o,i,f,g ; reference is i,f,g,o

_CACHE = {}


def build_kernel(T_steps=T):
    import concourse.bass as bass
    import concourse.bacc as bacc
    import concourse.mybir as mybir
    from concourse.tile import TileContext
    from concourse.masks import make_identity

    fp32 = mybir.dt.float32
    bf16 = mybir.dt.bfloat16
    AF = mybir.ActivationFunctionType
    ALU = mybir.AluOpType

    TB = T_steps * BL
    MT = TB // 128  # phase-1 m-tiles

    nc = bacc.Bacc()
    xT = nc.declare_dram_parameter("xT", [4, 128, TB], bf16, isOutput=False)
    wit = nc.declare_dram_parameter("wit", [4, 128, G4], bf16, isOutput=False)
    wht = nc.declare_dram_parameter("wht", [4, 128, G4], bf16, isOutput=False)
    brow = nc.declare_dram_parameter("brow", [1, G4], bf16, isOutput=False)
    h0t = nc.declare_dram_parameter("h0t", [128, 112], bf16, isOutput=False)
    c0l = nc.declare_dram_parameter("c0l", [112, 128], fp32, isOutput=False)
    ys = nc.declare_dram_parameter("ys", [T_steps, 4, BL, 128], fp32, isOutput=True)
    Z = nc.dram_tensor("zscratch", [TB, G4], bf16, kind="Internal")

    with TileContext(nc) as tc:
        with (
            tc.tile_pool(name="const", bufs=1) as constp,
            tc.tile_pool(name="wpool", bufs=1) as wpool,
            tc.tile_pool(name="state", bufs=1) as statep,
        ):
            ident112 = constp.tile([112, 112], fp32)
            make_identity(nc, ident112[:, :])
            identI = constp.tile([16, 16], bf16)
            make_identity(nc, identI[:, :])
            ones1 = constp.tile([1, 128], bf16)
            nc.gpsimd.memset(ones1[:, :], 1.0)
            browsb = constp.tile([1, G4], bf16)
            nc.sync.dma_start(out=browsb[:, :], in_=brow[:, :])

            whsb = [wpool.tile([128, G4], bf16, tag=f"wh{k}", name=f"wh{k}")
                    for k in range(4)]
            wisb = [wpool.tile([128, G4], bf16, tag=f"wi{k}", name=f"wi{k}")
                    for k in range(4)]
            for k in range(4):
                nc.sync.dma_start(out=whsb[k][:, :], in_=wht[k])
                nc.sync.dma_start(out=wisb[k][:, :], in_=wit[k])

            # State: hT4 cols 32k:32k+16 = stationary for h-block k; C rows
            # 32s:32s+16 = c for h-block s (garbage in the gap rows is inert).
            hT4 = statep.tile([128, 112], bf16, tag="hT4")
            C = statep.tile([112, 128], fp32, tag="C")
            nc.sync.dma_start(out=hT4[:, :], in_=h0t[:, :])
            nc.sync.dma_start(out=C[:, :], in_=c0l[:, :])

            # ---- Phase 1: Z = x @ Wi.T + b (all t), bf16 out to DRAM ----
            with (
                tc.tile_pool(name="xin", bufs=4) as xinp,
                tc.tile_pool(name="zps", bufs=2, space="PSUM") as zpsp,
                tc.tile_pool(name="zst", bufs=3) as zstp,
            ):
                for m in range(MT):
                    xk = [xinp.tile([128, 128], bf16, tag=f"x{k}", name=f"x{k}")
                          for k in range(4)]
                    for k in range(4):
                        nc.sync.dma_start(
                            out=xk[k][:, :], in_=xT[k, :, 128 * m:128 * (m + 1)])
                    zp = zpsp.tile([128, G4], fp32, tag="zp")
                    for s in range(4):
                        sl = slice(512 * s, 512 * (s + 1))
                        nc.tensor.matmul(zp[:, sl], ones1[0:1, :], browsb[0:1, sl],
                                         start=True, stop=False)
                        for k in range(4):
                            nc.tensor.matmul(zp[:, sl], xk[k][:, :], wisb[k][:, sl],
                                             start=False, stop=(k == 3))
                    zs = zstp.tile([128, G4], bf16, tag="zs")
                    if m % 2 == 0:
                        nc.vector.tensor_copy(zs[:, :], zp[:, :])
                    else:
                        nc.scalar.copy(zs[:, :], zp[:, :])
                    nc.sync.dma_start(out=Z[128 * m:128 * (m + 1), :], in_=zs[:, :])

            # ---- Phase 2: recurrent scan ----
            with (
                tc.tile_pool(name="zin", bufs=3) as zinp,
                tc.tile_pool(name="gps", bufs=2, space="PSUM") as gpsp,
                tc.tile_pool(name="ptp", bufs=2, space="PSUM") as ptp,
                tc.tile_pool(name="junk", bufs=2, space="PSUM") as junkp,
                tc.tile_pool(name="work", bufs=2) as workp,
            ):
                for t in range(T_steps):
                    zsb = zinp.tile([BL, G4], bf16, tag="zsb")
                    nc.sync.dma_start(out=zsb[:, :], in_=Z[t * BL:(t + 1) * BL, :])

                    G = gpsp.tile([112, 512], fp32, tag="G")
                    for s in range(4):
                        nc.tensor.matmul(
                            G[32 * s:32 * s + 16, :], identI[:, :],
                            zsb[:, 512 * s:512 * (s + 1)],
                            start=True, stop=False, tile_position=(0, 32 * s),
                            skip_group_check=True)
                    for k in range(4):
                        for s in range(4):
                            nc.tensor.matmul(
                                G[32 * s:32 * s + 16, :], hT4[:, 32 * k:32 * k + 16],
                                whsb[k][:, 512 * s:512 * (s + 1)],
                                start=False, stop=(k == 3), tile_position=(0, 32 * s),
                                skip_group_check=True)

                    # free layout is [o|i|f|g]x128: sigmoid(i,f,g) first —
                    # it heads the critical path; sigmoid(o) runs under the
                    # DVE work (g columns pre-scaled x2 on host).
                    A = workp.tile([112, 512], fp32, tag="A")
                    nc.scalar.activation(A[:, 128:512], G[0:112, 128:512],
                                         AF.Sigmoid)
                    nc.scalar.activation(A[:, 0:128], G[0:112, 0:128],
                                         AF.Sigmoid)
                    # g~ = tanh = 2*sigmoid(2x) - 1 (gpsimd, off DVE's back)
                    nc.gpsimd.tensor_scalar(A[:, 384:512], A[:, 384:512],
                                            2.0, -1.0, ALU.mult, ALU.add)
                    T2 = workp.tile([112, 128], fp32, tag="T2")
                    nc.vector.tensor_mul(T2[:, :], A[:, 256:384], C[:, :])
                    T1 = workp.tile([112, 128], fp32, tag="T1")
                    nc.vector.tensor_mul(T1[:, :], A[:, 128:256], A[:, 384:512])
                    nc.vector.tensor_add(C[:, :], T1[:, :], T2[:, :])
                    TC = workp.tile([112, 128], fp32, tag="TC")
                    nc.scalar.activation(TC[:, :], C[:, :], AF.Tanh)
                    hsb = workp.tile([112, 128], fp32, tag="hsb")
                    nc.vector.tensor_mul(hsb[:, :], A[:, 0:128], TC[:, :])
                    # HAM warm-keepers: tiny matmuls whose inputs become
                    # ready mid-ewise keep PE activity in every HAM window
                    # so the real matmuls run at 2.4 GHz.
                    JK = junkp.tile([16, 64], fp32, tag="JK")
                    nc.tensor.matmul(JK[:, :], ident112[0:16, 0:16],
                                     A[0:16, 0:64], start=True, stop=True,
                                     skip_group_check=True)
                    JK2 = junkp.tile([16, 64], fp32, tag="JK2")
                    nc.tensor.matmul(JK2[:, :], ident112[0:16, 0:16],
                                     TC[0:16, 0:64], start=True, stop=True,
                                     skip_group_check=True)

                    for s in range(4):
                        nc.sync.dma_start(out=ys[t, s],
                                          in_=hsb[32 * s:32 * s + 16, :])

                    PT = ptp.tile([128, 112], fp32, tag="PT")
                    nc.tensor.transpose(PT[:, :], hsb[0:112, :], ident112[:, :])
                    nc.vector.tensor_copy(hT4[:, :], PT[:, :])

    nc.finalize()
    return nc


def _prep_w(Wi, Wh, b):
    """Reference (4H,K) weights -> device [4,128,4H] bf16 transposed chunks
    with columns ordered (h-block s, gate i/f/o/g, jj) and g scaled x2."""
    def cols(W):
        W = np.asarray(W, np.float32).reshape(4, 4, 128, -1)  # [gref, s, jj, K]
        W = W[GPERM]                 # -> device gate order i,f,o,g
        W[3] *= 2.0                  # g pre-scale for tanh-via-sigmoid
        W = W.transpose(1, 0, 2, 3).reshape(G4, -1)  # [(s,gd,jj), K]
        Wt = np.ascontiguousarray(W.T)               # [K, 4H]
        return Wt.reshape(4, 128, G4).astype(bfloat16)

    bv = np.asarray(b, np.float32).reshape(4, 4, 128)[GPERM]
    bv[3] *= 2.0
    bv = bv.transpose(1, 0, 2).reshape(1, G4).astype(bfloat16)
    return cols(Wi), cols(Wh), bv


def _host_prep(inputs_emb, mask, h0, c0, Wi_f, Wh_f, b_f, Wi_b, Wh_b, b_b):
    x = np.asarray(inputs_emb, dtype=np.float32)
    mask = np.asarray(mask, dtype=np.float32)
    lengths = mask.astype(np.int32).sum(axis=1)
    t_idx = np.arange(T, dtype=np.int64)[None, :]
    P = (lengths[:, None].astype(np.int64) - 1 - t_idx) % T  # involution
    x_proc = np.take_along_axis(x, P[:, :, None], axis=1)

    wif, whf, bf_ = _prep_w(Wi_f, Wh_f, b_f)
    wib, whb, bb_ = _prep_w(Wi_b, Wh_b, b_b)
    h0 = np.asarray(h0, np.float32)
    c0 = np.asarray(c0, np.float32)

    in_maps = []
    for cidx in range(NCORES):
        d = cidx // 4
        sl = slice((cidx % 4) * BL, (cidx % 4 + 1) * BL)
        xd = (x if d == 0 else x_proc)[sl]  # [BL, T, D]
        # xT[k, :, t*BL + b] = xd[b, t, 128k:...]
        xTa = xd.transpose(2, 1, 0).reshape(4, 128, T, BL).reshape(4, 128, T * BL)
        h0a = np.zeros((128, 112), np.float32)
        c0a = np.zeros((112, 128), np.float32)
        for k in range(4):
            h0a[:, 32 * k:32 * k + 16] = h0[sl, 128 * k:128 * (k + 1)].T
            c0a[32 * k:32 * k + 16, :] = c0[sl, 128 * k:128 * (k + 1)]
        in_maps.append({
            "xT": np.ascontiguousarray(xTa).astype(bfloat16),
            "wit": wif if d == 0 else wib,
            "wht": whf if d == 0 else whb,
            "brow": bf_ if d == 0 else bb_,
            "h0t": h0a.astype(bfloat16),
            "c0l": c0a,
        })
    return in_maps, P


def _host_post(results, P):
    outs = []
    for r in results:
        y = r["ys"]  # [T, 4, BL, 128]
        outs.append(np.ascontiguousarray(y.transpose(2, 0, 1, 3)).reshape(BL, T, H))
    ys_f = np.concatenate(outs[:4], 0)  # [B, T, H]
    ys_b = np.concatenate(outs[4:], 0)
    out_b = np.take_along_axis(ys_b, P[:, :, None], axis=1)
    return np.concatenate([ys_f, out_b], axis=-1).astype(np.float32)


def kernel(**inputs):
    from concourse.bass_utils import run_bass_kernel_spmd
    in_maps, P = _host_prep(**inputs)
    if "nc" not in _CACHE:
        _CACHE["nc"] = build_kernel()
    nc = _CACHE["nc"]
    res = run_bass_kernel_spmd(nc, in_maps, list(range(NCORES)))
    return _host_post(res.results, P)


# revision 11
# speedup vs baseline: 3.6466x; 1.1237x over previous
"""BiLSTM (reference nn_CharBiGRU) Trainium2 Bass kernel, v2.

Strategy (8 cores = 2 directions x 4 batch-quarters, 16 rows each):
  - Phase 1: input projection Z = x @ Wi.T + b for ALL timesteps as one
    big bf16 GEMM (128-row (t,b) tiles, K=512, N=2048), staged to DRAM
    scratch in bf16.
  - Phase 2: per-step recurrence. Gates live in PSUM as [112, 512]:
    partition strip 32s:32s+16 = h-block s (j in [128s,128s+128)), free
    = gate-major [i|f|o|g] x 128 within the block. Z_t enters PSUM via a
    K=16 identity matmul; h @ Wh.T accumulates as 16 bf16 matmuls
    4-way column-tiled (tile_position=(0,32s)) so the four strips
    stream concurrently on separate XBUSes.
  - Elementwise keeps everything at matching partition bases: one
    sigmoid over all four gates (g pre-scaled by 2 on host so
    tanh(x) = 2*sigmoid(2x)-1), gpsimd fixes g, DVE does the cell
    update, one PE transpose of h [112,128] -> hT4 [128,112] yields all
    four next-step stationaries as column slices.
  - Backward direction runs on cores 4-7 over host-rotated x (same
    involution trick as the reference); host un-rotates its output.
"""

import numpy as np
from ml_dtypes import bfloat16

B, T, D, H = 64, 512, 512, 512
G4 = 4 * H
NCORES = 8
BL = 16  # batch rows per core
GPERM = [3, 0, 1, 2]  # device gate order o,i,f,g ; reference is i,f,g,o

_CACHE = {}


def build_kernel(T_steps=T):
    import concourse.bass as bass
    import concourse.bacc as bacc
    import concourse.mybir as mybir
    from concourse.tile import TileContext
    from concourse.masks import make_identity

    fp32 = mybir.dt.float32
    bf16 = mybir.dt.bfloat16
    AF = mybir.ActivationFunctionType
    ALU = mybir.AluOpType

    TB = T_steps * BL
    MT = TB // 128  # phase-1 m-tiles

    nc = bacc.Bacc()
    xT = nc.declare_dram_parameter("xT", [4, 128, TB], bf16, isOutput=False)
    wit = nc.declare_dram_parameter("wit", [4, 128, G4], bf16, isOutput=False)
    wht = nc.declare_dram_parameter("wht", [4, 128, G4], bf16, isOutput=False)
    brow = nc.declare_dram_parameter("brow", [1, G4], bf16, isOutput=False)
    h0t = nc.declare_dram_parameter("h0t", [128, 112], bf16, isOutput=False)
    c0l = nc.declare_dram_parameter("c0l", [112, 128], fp32, isOutput=False)
    ys = nc.declare_dram_parameter("ys", [T_steps, 4, BL, 128], fp32, isOutput=True)
    Z = nc.dram_tensor("zscratch", [TB, G4], bf16, kind="Internal")

    with TileContext(nc) as tc:
        with (
            tc.tile_pool(name="const", bufs=1) as constp,
            tc.tile_pool(name="wpool", bufs=1) as wpool,
            tc.tile_pool(name="state", bufs=1) as statep,
        ):
            ident112 = constp.tile([112, 112], fp32)
            make_identity(nc, ident112[:, :])
            identI = constp.tile([16, 16], bf16)
            make_identity(nc, identI[:, :])
            ones1 = constp.tile([1, 128], bf16)
            nc.gpsimd.memset(ones1[:, :], 1.0)
            browsb = constp.tile([1, G4], bf16)
            nc.sync.dma_start(out=browsb[:, :], in_=brow[:, :])

            whsb = [wpool.tile([128, G4], bf16, tag=f"wh{k}", name=f"wh{k}")
                    for k in range(4)]
            wisb = [wpool.tile([128, G4], bf16, tag=f"wi{k}", name=f"wi{k}")
                    for k in range(4)]
            for k in range(4):
                nc.sync.dma_start(out=whsb[k][:, :], in_=wht[k])
                nc.sync.dma_start(out=wisb[k][:, :], in_=wit[k])

            # State: hT4 cols 32k:32k+16 = stationary for h-block k; C rows
            # 32s:32s+16 = c for h-block s (garbage in the gap rows is inert).
            hT4 = statep.tile([128, 112], bf16, tag="hT4")
            C = statep.tile([112, 128], fp32, tag="C")
            nc.sync.dma_start(out=hT4[:, :], in_=h0t[:, :])
            nc.sync.dma_start(out=C[:, :], in_=c0l[:, :])

            # ---- Phase 1: Z = x @ Wi.T + b (all t), bf16 out to DRAM ----
            with (
                tc.tile_pool(name="xin", bufs=4) as xinp,
                tc.tile_pool(name="zps", bufs=2, space="PSUM") as zpsp,
                tc.tile_pool(name="zst", bufs=3) as zstp,
            ):
                for m in range(MT):
                    xk = [xinp.tile([128, 128], bf16, tag=f"x{k}", name=f"x{k}")
                          for k in range(4)]
                    for k in range(4):
                        nc.sync.dma_start(
                            out=xk[k][:, :], in_=xT[k, :, 128 * m:128 * (m + 1)])
                    zp = zpsp.tile([128, G4], fp32, tag="zp")
                    for s in range(4):
                        sl = slice(512 * s, 512 * (s + 1))
                        nc.tensor.matmul(zp[:, sl], ones1[0:1, :], browsb[0:1, sl],
                                         start=True, stop=False)
                        for k in range(4):
                            nc.tensor.matmul(zp[:, sl], xk[k][:, :], wisb[k][:, sl],
                                             start=False, stop=(k == 3))
                    zs = zstp.tile([128, G4], bf16, tag="zs")
                    if m % 2 == 0:
                        nc.vector.tensor_copy(zs[:, :], zp[:, :])
                    else:
                        nc.scalar.copy(zs[:, :], zp[:, :])
                    nc.sync.dma_start(out=Z[128 * m:128 * (m + 1), :], in_=zs[:, :])

            # ---- Phase 2: recurrent scan ----
            with (
                tc.tile_pool(name="zin", bufs=3) as zinp,
                tc.tile_pool(name="gps", bufs=2, space="PSUM") as gpsp,
                tc.tile_pool(name="ptp", bufs=2, space="PSUM") as ptp,
                tc.tile_pool(name="junk", bufs=1, space="PSUM") as junkp,
                tc.tile_pool(name="work", bufs=2) as workp,
            ):
                for t in range(T_steps):
                    zsb = zinp.tile([BL, G4], bf16, tag="zsb")
                    nc.sync.dma_start(out=zsb[:, :], in_=Z[t * BL:(t + 1) * BL, :])

                    # Gates split across two PSUM banks so the f,g half can
                    # be read by ACT while the o,i half is still streaming:
                    # Ga cols 0:256 = strip cols 256:512 (f,g), Gb cols
                    # 0:256 = strip cols 0:256 (o,i).
                    Ga = gpsp.tile([112, 512], fp32, tag="Ga")
                    Gb = gpsp.tile([112, 512], fp32, tag="Gb")
                    for s in range(4):
                        nc.tensor.matmul(
                            Ga[32 * s:32 * s + 16, 0:256], identI[:, :],
                            zsb[:, 512 * s + 256:512 * s + 512],
                            start=True, stop=False, tile_position=(0, 32 * s),
                            skip_group_check=True)
                        nc.tensor.matmul(
                            Gb[32 * s:32 * s + 16, 0:256], identI[:, :],
                            zsb[:, 512 * s:512 * s + 256],
                            start=True, stop=False, tile_position=(0, 32 * s),
                            skip_group_check=True)
                    for k in range(4):
                        for s in range(4):
                            nc.tensor.matmul(
                                Ga[32 * s:32 * s + 16, 0:256],
                                hT4[:, 32 * k:32 * k + 16],
                                whsb[k][:, 512 * s + 256:512 * s + 512],
                                start=False, stop=(k == 3),
                                tile_position=(0, 32 * s),
                                skip_group_check=True)
                    for k in range(4):
                        for s in range(4):
                            nc.tensor.matmul(
                                Gb[32 * s:32 * s + 16, 0:256],
                                hT4[:, 32 * k:32 * k + 16],
                                whsb[k][:, 512 * s:512 * s + 256],
                                start=False, stop=(k == 3),
                                tile_position=(0, 32 * s),
                                skip_group_check=True)

                    # free layout of A is [o|i|f|g]x128: sigmoid(f,g) first
                    # (they head the critical path and their bank finishes
                    # first); sigmoid(i) then sigmoid(o) follow as the o,i
                    # bank completes (g columns pre-scaled x2 on host).
                    A = workp.tile([112, 512], fp32, tag="A")
                    nc.scalar.activation(A[:, 256:512], Ga[0:112, 0:256],
                                         AF.Sigmoid)
                    nc.scalar.activation(A[:, 128:256], Gb[0:112, 128:256],
                                         AF.Sigmoid)
                    nc.scalar.activation(A[:, 0:128], Gb[0:112, 0:128],
                                         AF.Sigmoid)
                    # g~ = tanh = 2*sigmoid(2x) - 1 (gpsimd, off DVE's back)
                    nc.gpsimd.tensor_scalar(A[:, 384:512], A[:, 384:512],
                                            2.0, -1.0, ALU.mult, ALU.add)
                    T2 = workp.tile([112, 128], fp32, tag="T2")
                    nc.vector.tensor_mul(T2[:, :], A[:, 256:384], C[:, :])
                    T1 = workp.tile([112, 128], fp32, tag="T1")
                    nc.vector.tensor_mul(T1[:, :], A[:, 128:256], A[:, 384:512])
                    nc.vector.tensor_add(C[:, :], T1[:, :], T2[:, :])
                    TC = workp.tile([112, 128], fp32, tag="TC")
                    nc.scalar.activation(TC[:, :], C[:, :], AF.Tanh)
                    hsb = workp.tile([112, 128], fp32, tag="hsb")
                    nc.vector.tensor_mul(hsb[:, :], A[:, 0:128], TC[:, :])
                    # HAM warm-keepers: tiny matmuls whose inputs become
                    # ready mid-ewise keep PE activity in every HAM window
                    # so the real matmuls run at 2.4 GHz.
                    JK = junkp.tile([16, 64], fp32, tag="JK")
                    nc.tensor.matmul(JK[:, :], ident112[0:16, 0:16],
                                     A[0:16, 0:64], start=True, stop=True,
                                     skip_group_check=True)
                    JK2 = junkp.tile([16, 64], fp32, tag="JK2")
                    nc.tensor.matmul(JK2[:, :], ident112[0:16, 0:16],
                                     TC[0:16, 0:64], start=True, stop=True,
                                     skip_group_check=True)

                    for s in range(4):
                        nc.sync.dma_start(out=ys[t, s],
                                          in_=hsb[32 * s:32 * s + 16, :])

                    PT = ptp.tile([128, 112], fp32, tag="PT")
                    nc.tensor.transpose(PT[:, :], hsb[0:112, :], ident112[:, :])
                    # split the cast so next step's k=0 matmuls start as
                    # soon as their stationary slice (cols 0:32) lands
                    nc.vector.tensor_copy(hT4[:, 0:32], PT[:, 0:32])
                    nc.vector.tensor_copy(hT4[:, 32:112], PT[:, 32:112])

    nc.finalize()
    return nc


def _prep_w(Wi, Wh, b):
    """Reference (4H,K) weights -> device [4,128,4H] bf16 transposed chunks
    with columns ordered (h-block s, gate i/f/o/g, jj) and g scaled x2."""
    def cols(W):
        W = np.asarray(W, np.float32).reshape(4, 4, 128, -1)  # [gref, s, jj, K]
        W = W[GPERM]                 # -> device gate order i,f,o,g
        W[3] *= 2.0                  # g pre-scale for tanh-via-sigmoid
        W = W.transpose(1, 0, 2, 3).reshape(G4, -1)  # [(s,gd,jj), K]
        Wt = np.ascontiguousarray(W.T)               # [K, 4H]
        return Wt.reshape(4, 128, G4).astype(bfloat16)

    bv = np.asarray(b, np.float32).reshape(4, 4, 128)[GPERM]
    bv[3] *= 2.0
    bv = bv.transpose(1, 0, 2).reshape(1, G4).astype(bfloat16)
    return cols(Wi), cols(Wh), bv


def _host_prep(inputs_emb, mask, h0, c0, Wi_f, Wh_f, b_f, Wi_b, Wh_b, b_b):
    x = np.asarray(inputs_emb, dtype=np.float32)
    mask = np.asarray(mask, dtype=np.float32)
    lengths = mask.astype(np.int32).sum(axis=1)
    t_idx = np.arange(T, dtype=np.int64)[None, :]
    P = (lengths[:, None].astype(np.int64) - 1 - t_idx) % T  # involution
    x_proc = np.take_along_axis(x, P[:, :, None], axis=1)

    wif, whf, bf_ = _prep_w(Wi_f, Wh_f, b_f)
    wib, whb, bb_ = _prep_w(Wi_b, Wh_b, b_b)
    h0 = np.asarray(h0, np.float32)
    c0 = np.asarray(c0, np.float32)

    in_maps = []
    for cidx in range(NCORES):
        d = cidx // 4
        sl = slice((cidx % 4) * BL, (cidx % 4 + 1) * BL)
        xd = (x if d == 0 else x_proc)[sl]  # [BL, T, D]
        # xT[k, :, t*BL + b] = xd[b, t, 128k:...]
        xTa = xd.transpose(2, 1, 0).reshape(4, 128, T, BL).reshape(4, 128, T * BL)
        h0a = np.zeros((128, 112), np.float32)
        c0a = np.zeros((112, 128), np.float32)
        for k in range(4):
            h0a[:, 32 * k:32 * k + 16] = h0[sl, 128 * k:128 * (k + 1)].T
            c0a[32 * k:32 * k + 16, :] = c0[sl, 128 * k:128 * (k + 1)]
        in_maps.append({
            "xT": np.ascontiguousarray(xTa).astype(bfloat16),
            "wit": wif if d == 0 else wib,
            "wht": whf if d == 0 else whb,
            "brow": bf_ if d == 0 else bb_,
            "h0t": h0a.astype(bfloat16),
            "c0l": c0a,
        })
    return in_maps, P


def _host_post(results, P):
    outs = []
    for r in results:
        y = r["ys"]  # [T, 4, BL, 128]
        outs.append(np.ascontiguousarray(y.transpose(2, 0, 1, 3)).reshape(BL, T, H))
    ys_f = np.concatenate(outs[:4], 0)  # [B, T, H]
    ys_b = np.concatenate(outs[4:], 0)
    out_b = np.take_along_axis(ys_b, P[:, :, None], axis=1)
    return np.concatenate([ys_f, out_b], axis=-1).astype(np.float32)


def kernel(**inputs):
    from concourse.bass_utils import run_bass_kernel_spmd
    in_maps, P = _host_prep(**inputs)
    if "nc" not in _CACHE:
        _CACHE["nc"] = build_kernel()
    nc = _CACHE["nc"]
    res = run_bass_kernel_spmd(nc, in_maps, list(range(NCORES)))
    return _host_post(res.results, P)
